# revision 16
# baseline (speedup 1.0000x reference)
"""Trainium2 Bass kernel for a dense transformer block (B=4,T=1024,H=1024,NH=16,FF=4096).

Sharding: 8 cores = (batch b, token-half h). Each core computes the full block
for its 512 query tokens; K/V projections are computed over all 1024 tokens of
the batch on each core (no cross-core collectives).

Device layout is fully "transposed": activations live as [feature->partitions,
token->free] SBUF tiles. LayerNorm/softmax reductions over features/keys become
TensorE ones-matmuls (fused reduce+broadcast). Token-half cores are made
SPMD-uniform by rotating core h=0's xT columns by 512 so query tokens are
always xT columns 512:1024; all per-core differences (masks, RoPE tables) are
inputs. The reference's softmax-then-multiplicative-mask semantics are kept:
exp over all keys feeds the denominator, masked exp feeds the AV matmul.
"""
import sys
sys.path.insert(0, "/opt/trn_rl_repo")
import numpy as np
import ml_dtypes

B, T, H, NH = 4, 1024, 1024, 16
HS = H // NH          # 64
FF = 4 * H            # 4096
EPS = 1e-5
P = 128
TQ = T // 2           # 512 query tokens per core
NT = T // P           # 8 feature/token tiles
NFF = FF // P         # 32
NCORES = 8

_bf16 = ml_dtypes.bfloat16


# ----------------------------------------------------------------------------
# device program
# ----------------------------------------------------------------------------

def build(repeat=1, debug_outputs=False):
    import concourse.bass as bass
    import concourse.mybir as mybir
    import concourse.tile as tile
    from concourse import bacc
    from contextlib import ExitStack

    f32 = mybir.dt.float32
    bf = mybir.dt.bfloat16
    AF = mybir.ActivationFunctionType
    ALU = mybir.AluOpType

    nc = bacc.Bacc("TRN2", target_bir_lowering=False, debug=False,
                   num_devices=NCORES)

    def din(name, shape, dt=f32):
        return nc.dram_tensor(name, shape, dt, kind="ExternalInput").ap()

    # per-core inputs
    xT_lo = din("xT_lo", [H, TQ], bf)        # x^T columns 0:512 (bf16)
    xT_hi = din("xT_hi", [H, TQ])            # x^T columns 512:1024 (= query tokens)
    wq = din("wq", [H, H], bf)               # [h_in, f_out], rope-permuted cols
    wk = din("wk", [H, H], bf)
    wv = din("wv", [H, H], bf)
    wo = din("wo", [H, H], bf)
    wfc = din("wfc", [H, FF], bf)
    wpr = din("wpr", [FF, H], bf)
    bq = din("bq", [P, NT])                  # permuted, [partition, tile]
    bk = din("bk", [P, NT])
    bo = din("bo", [P, NT])
    bpr = din("bpr", [P, NT])
    bfc = din("bfc", [P, NFF])
    ln1w = din("ln1w", [P, NT])
    ln1b = din("ln1b", [P, NT])
    ln2w = din("ln2w", [P, NT])
    ln2b = din("ln2b", [P, NT])
    bvb = din("bvb", [P, H])                 # bv broadcast across partitions
    cosK = din("cosK", [P, T], bf)           # rope tables, xT column order
    ssgnK = din("ssgnK", [P, T], bf)         # +sin rows j=0, -sin rows j=1
    mask_lo = din("mask_lo", [P, TQ], bf)    # kt 0..3 mask (all-0 or all-1)
    mscal = din("mscal", [P, 1])             # kt 0..3 mask as scalar (0.0 or 1.0)
    mask_hi = din("mask_hi", [P, 4, TQ], bf) # kt 4..7 triangular masks

    outT = nc.dram_tensor("outT", [H, TQ], f32, kind="ExternalOutput").ap()
    dbg = {}
    if debug_outputs:
        for name, shape in [("d_h1T", [H, T]), ("d_kT", [H, T]), ("d_qT", [H, TQ]),
                            ("d_v", [T, H]), ("d_attnT", [H, TQ]),
                            ("d_x2T", [H, TQ]), ("d_mT", [FF, TQ])]:
            dbg[name] = nc.dram_tensor(name, shape, f32, kind="ExternalOutput").ap()

    def body(tc, const1, ones_bf):
        with ExitStack() as ctx:
            # ------------ long-lived pools for this block iteration ----------
            persist = ctx.enter_context(tc.tile_pool(name="persist", bufs=1))
            x_hi = persist.tile([P, NT, TQ], f32, tag="x_hi")
            x2T = persist.tile([P, NT, TQ], f32, tag="x2T")
            biases = persist.tile([P, 2 * NT], f32, tag="biases")  # bq|bk
            bo_sb = persist.tile([P, NT], f32, tag="bo_sb")
            bpr_sb = persist.tile([P, NT], f32, tag="bpr_sb")
            bfc_sb = persist.tile([P, NFF], f32, tag="bfc_sb")
            ln_sb = persist.tile([P, 4 * NT], f32, tag="ln_sb")  # ln1w|ln1b|ln2w|ln2b
            eps_sb = persist.tile([P, 1], f32, tag="eps_sb")
            nc.vector.memset(eps_sb, EPS)
            msc_sb = persist.tile([P, 1], f32, tag="msc_sb")
            nc.sync.dma_start(out=msc_sb, in_=mscal)

            nc.sync.dma_start(out=biases[:, 0:NT], in_=bq)
            nc.sync.dma_start(out=biases[:, NT:2 * NT], in_=bk)
            nc.sync.dma_start(out=bo_sb, in_=bo)
            nc.sync.dma_start(out=bpr_sb, in_=bpr)
            nc.sync.dma_start(out=bfc_sb, in_=bfc)
            nc.sync.dma_start(out=ln_sb[:, 0:NT], in_=ln1w)
            nc.sync.dma_start(out=ln_sb[:, NT:2 * NT], in_=ln1b)
            nc.sync.dma_start(out=ln_sb[:, 2 * NT:3 * NT], in_=ln2w)
            nc.sync.dma_start(out=ln_sb[:, 3 * NT:4 * NT], in_=ln2b)
            nc.sync.dma_start(out=x_hi, in_=xT_hi.rearrange("(n p) t -> p n t", p=P))

            with ExitStack() as attn_scope:
                aacts = attn_scope.enter_context(tc.tile_pool(name="aacts", bufs=1))
                h1T = aacts.tile([P, NT, T], bf, tag="h1T")
                kT = aacts.tile([P, NT, T], bf, tag="kT")
                qT = aacts.tile([P, NT, TQ], bf, tag="qT")
                vsb = aacts.tile([P, NT, H], bf, tag="vsb")     # [tok_p, kt, feat]
                attnT = aacts.tile([P, NT, TQ], bf, tag="attnT")
                # [V'|ones] fused stationary operand for kt 0..3:
                # layout [p, kt, head, {V',ones}, hs]
                vaug = aacts.tile([P, 4, NH, 2, HS], bf, tag="vaug")
                bvb_sb = aacts.tile([P, H], f32, tag="bvb_sb")
                cos_sb = aacts.tile([P, T], bf, tag="cos_sb")
                ssgn_sb = aacts.tile([P, T], bf, tag="ssgn_sb")
                mhi_sb = aacts.tile([P, 4, TQ], bf, tag="mhi_sb")

                nc.sync.dma_start(out=bvb_sb, in_=bvb)
                nc.sync.dma_start(out=cos_sb, in_=cosK)
                nc.sync.dma_start(out=ssgn_sb, in_=ssgnK)
                nc.sync.dma_start(out=mhi_sb, in_=mask_hi)

                # ---------------- phase 1: LN1 (transposed layernorm) --------
                with ExitStack() as p1:
                    x_lo_p = p1.enter_context(tc.tile_pool(name="x_lo", bufs=1))
                    stat = p1.enter_context(tc.tile_pool(name="stat", bufs=1))
                    tmpp = p1.enter_context(tc.tile_pool(name="ln_tmp", bufs=3))
                    psln = p1.enter_context(tc.tile_pool(name="psln", bufs=1, space="PSUM"))

                    x_lo = x_lo_p.tile([P, NT, TQ], bf, tag="x_lo")
                    nc.sync.dma_start(out=x_lo, in_=xT_lo.rearrange("(n p) t -> p n t", p=P))

                    mu_ps = psln.tile([P, T], f32, tag="ln_mu")
                    sq_ps = psln.tile([P, T], f32, tag="ln_sq")
                    for kt in range(NT):
                        xbf_hi = tmpp.tile([P, TQ], bf, tag="xbf_hi")
                        nc.scalar.activation(xbf_hi, x_hi[:, kt, :], AF.Copy)
                        sq_lo = tmpp.tile([P, TQ], bf, tag="sq_lo")
                        nc.vector.tensor_mul(sq_lo, x_lo[:, kt, :], x_lo[:, kt, :])
                        sq_hi = tmpp.tile([P, TQ], bf, tag="sq_hi")
                        nc.vector.tensor_mul(sq_hi, xbf_hi, xbf_hi)
                        first, last = (kt == 0), (kt == NT - 1)
                        nc.tensor.matmul(mu_ps[:, 0:TQ], ones_bf, x_lo[:, kt, :],
                                         start=first, stop=last)
                        nc.tensor.matmul(mu_ps[:, TQ:T], ones_bf, xbf_hi,
                                         start=first, stop=last)
                        nc.tensor.matmul(sq_ps[:, 0:TQ], ones_bf, sq_lo,
                                         start=first, stop=last)
                        nc.tensor.matmul(sq_ps[:, TQ:T], ones_bf, sq_hi,
                                         start=first, stop=last)

                    mu_sb = stat.tile([P, T], f32, tag="mu_sb")
                    rstd = stat.tile([P, T], f32, tag="rstd")
                    tmp1 = stat.tile([P, T], f32, tag="lntmp1")
                    nc.scalar.activation(mu_sb, mu_ps, AF.Copy, scale=1.0 / H)
                    nc.vector.tensor_mul(tmp1, mu_sb, mu_sb)
                    nc.vector.scalar_tensor_tensor(tmp1, sq_ps, 1.0 / H, tmp1,
                                                   ALU.mult, ALU.subtract)
                    nc.scalar.activation(tmp1, tmp1, AF.Sqrt, bias=eps_sb)
                    nc.vector.reciprocal(rstd, tmp1)

                    for kt in range(NT):
                        for blk, xpart in ((0, x_lo[:, kt, :]), (1, x_hi[:, kt, :])):
                            sl = slice(blk * TQ, (blk + 1) * TQ)
                            t = tmpp.tile([P, TQ], f32, tag="h1tmp")
                            nc.vector.tensor_sub(t, xpart, mu_sb[:, sl])
                            nc.vector.tensor_mul(t, t, rstd[:, sl])
                            nc.scalar.activation(h1T[:, kt, sl], t, AF.Identity,
                                                 bias=ln_sb[:, NT + kt:NT + kt + 1],
                                                 scale=ln_sb[:, kt:kt + 1])
                    if debug_outputs:
                        dT = dbg["d_h1T"].rearrange("(n p) t -> p n t", p=P)
                        for kt in range(NT):
                            c = tmpp.tile([P, T], f32, tag="dbgc")
                            nc.vector.tensor_copy(c, h1T[:, kt, :])
                            nc.sync.dma_start(out=dT[:, kt, :], in_=c)

                # -------- phase 2+3: QKV/RoPE interleaved with attention ------
                with ExitStack() as p23:
                    wpool = p23.enter_context(tc.tile_pool(name="wqkv", bufs=3))
                    rtmp = p23.enter_context(tc.tile_pool(name="rtmp", bufs=2))
                    epool = p23.enter_context(tc.tile_pool(
                        name="epool", bufs=1 if debug_outputs else 2))
                    dpool = p23.enter_context(tc.tile_pool(name="dpool", bufs=1))
                    ps23 = p23.enter_context(tc.tile_pool(name="ps23", bufs=3, space="PSUM"))
                    ps_av = p23.enter_context(tc.tile_pool(name="ps_av", bufs=2, space="PSUM"))

                    def grp_psum(name):
                        t = ps23.tile([P, 2, TQ], f32, tag="sp", name=name)
                        return t

                    # V in token-partition layout: [tok, feat]
                    wv_sb = wpool.tile([P, NT, H], bf, tag="wv_all", bufs=1)
                    nc.sync.dma_start(out=wv_sb, in_=wv.rearrange("(n p) m -> p n m", p=P))
                    for tt in range(NT):
                        for fb in range(2):
                            fsl = slice(fb * TQ, (fb + 1) * TQ)
                            ps = grp_psum(f"vps{tt}_{fb}")
                            for kt in range(NT):
                                nc.tensor.matmul(
                                    ps[:, 0, :], h1T[:, kt, tt * P:(tt + 1) * P],
                                    wv_sb[:, kt, fsl],
                                    start=(kt == 0), stop=(kt == NT - 1))
                            nc.vector.tensor_add(vsb[:, tt, fsl], ps[:, 0, :], bvb_sb[:, fsl])

                    nc.gpsimd.memset(vaug[:, :, :, 1, :], 1.0)
                    for kt in range(4):
                        nc.vector.tensor_scalar_mul(
                            vaug[:, kt, :, 0, :],
                            vsb[:, kt, :].rearrange("p (h d) -> p h d", h=NH),
                            msc_sb)

                    def proj_rope_tile(which, fo):
                        """Project one 128-feature tile of K^T (which=0) or Q^T."""
                        wdram = wk if which == 0 else wq
                        bias_off = NT if which == 0 else 0
                        cols = slice(0, T) if which == 0 else slice(TQ, T)
                        nblk = (cols.stop - cols.start) // TQ
                        wt = wpool.tile([P, NT, P], bf, tag="wqkv", name=f"w{which}_{fo}")
                        nc.sync.dma_start(
                            out=wt,
                            in_=wdram[:, fo * P:(fo + 1) * P]
                            .rearrange("(n p) m -> p n m", p=P))
                        for blk in range(nblk):
                            sl = slice(cols.start + blk * TQ, cols.start + (blk + 1) * TQ)
                            osl = slice(blk * TQ, (blk + 1) * TQ)
                            ps = grp_psum(f"qkp{which}_{fo}_{blk}")
                            for kt in range(NT):
                                nc.tensor.matmul(ps[:, 0, :], wt[:, kt, :], h1T[:, kt, sl],
                                                 start=(kt == 0), stop=(kt == NT - 1))
                            # rope: r = (ps+b)*cos + swap32((ps+b)*ssgn)
                            braw = rtmp.tile([P, TQ], bf, tag="braw", name=f"braw{which}_{fo}_{blk}")
                            bcol = biases[:, bias_off + fo:bias_off + fo + 1]
                            nc.scalar.activation(braw, ps[:, 0, :], AF.Identity, bias=bcol)
                            t1 = rtmp.tile([P, TQ], bf, tag="ropet1", name=f"t1_{which}_{fo}_{blk}")
                            t2 = rtmp.tile([P, TQ], bf, tag="ropet2", name=f"t2_{which}_{fo}_{blk}")
                            nc.vector.tensor_mul(t1, braw, ssgn_sb[:, sl])
                            nc.vector.tensor_mul(t2, braw, cos_sb[:, sl])
                            t1s = rtmp.tile([P, TQ], bf, tag="ropet1s", name=f"t1s_{which}_{fo}_{blk}")
                            for q in range(4):
                                src = q * 32 + (32 if q % 2 == 0 else -32)
                                nc.sync.dma_start(
                                    out=t1s[q * 32:(q + 1) * 32, :],
                                    in_=t1[src:src + 32, :])
                            dst = kT[:, fo, osl] if which == 0 else qT[:, fo, osl]
                            nc.vector.tensor_add(dst, t2, t1s)

                    for hp in range(NH // 2):
                        hd0, hd1 = 2 * hp, 2 * hp + 1
                        ft = hp
                        proj_rope_tile(0, hp)
                        proj_rope_tile(1, hp)
                        e_pair = epool.tile([P, NT, 2, TQ], bf, tag="e_pair", name=f"ep{hp}")
                        # row-packed score matmuls for the head pair (K=64 each)
                        for kt in range(NT):
                            ks = slice(kt * P, (kt + 1) * P)
                            sp = grp_psum(f"sp{hp}_{kt}")
                            nc.tensor.matmul(sp[:, 0, :], kT[0:HS, ft, ks],
                                             qT[0:HS, ft, :],
                                             start=True, stop=True,
                                             tile_position=(0, 0))
                            nc.tensor.matmul(sp[:, 1, :], kT[HS:P, ft, ks],
                                             qT[HS:P, ft, :],
                                             start=True, stop=True,
                                             tile_position=(64, 0))
                            nc.scalar.activation(e_pair[:, kt, :, :], sp, AF.Exp,
                                                 scale=1.0 / np.sqrt(HS))
                        for hd, j in ((hd0, 0), (hd1, 1)):
                            fp = (hd % 2) * HS
                            av2 = ps_av.tile([P, TQ], f32, tag="av2", name=f"av2_{hd}")
                            # kt 0..3: fused [V'|ones] -> rows 0:64 AV, 64:128 D
                            for kt in range(4):
                                nc.tensor.matmul(av2, vaug[:, kt, hd, :, :],
                                                 e_pair[:, kt, j, :],
                                                 start=(kt == 0), stop=False,
                                                 skip_group_check=True)
                            # kt 4..7: col-packed D (unmasked) + AV (masked)
                            em = epool.tile([P, 4, TQ], bf, tag="em", name=f"em{hd}", bufs=2)
                            for i in range(4):
                                nc.vector.tensor_mul(em[:, i, :],
                                                     e_pair[:, 4 + i, j, :],
                                                     mhi_sb[:, i, :])
                            for i in range(4):
                                kt = 4 + i
                                nc.tensor.matmul(av2[HS:P, :], ones_bf[:, 0:HS],
                                                 e_pair[:, kt, j, :],
                                                 start=False, stop=False,
                                                 tile_position=(0, 64),
                                                 skip_group_check=True)
                                nc.tensor.matmul(av2[0:HS, :],
                                                 vsb[:, kt, hd * HS:(hd + 1) * HS],
                                                 em[:, i, :],
                                                 start=False, stop=(kt == NT - 1),
                                                 tile_position=(0, 0),
                                                 skip_group_check=True)
                            rec = dpool.tile([P, TQ], f32, tag="rec", name=f"rec{hd}", bufs=2)
                            nc.vector.reciprocal(rec[0:HS, :], av2[HS:P, :])
                            nc.vector.tensor_mul(attnT[fp:fp + HS, ft, :],
                                                 av2[0:HS, :], rec[0:HS, :])

                    if debug_outputs:
                        for nm, srcT, width in (("d_kT", kT, T), ("d_qT", qT, TQ),
                                                ("d_v", vsb, H)):
                            dT = dbg[nm].rearrange("(n p) t -> p n t", p=P)
                            for kt in range(NT):
                                c = rtmp.tile([P, T], f32, tag="dbgc2", name=f"c{nm}_{kt}", bufs=1)
                                nc.vector.tensor_copy(c[:, 0:width], srcT[:, kt, :])
                                nc.sync.dma_start(out=dT[:, kt, :], in_=c[:, 0:width])
                        dT = dbg["d_attnT"].rearrange("(n p) t -> p n t", p=P)
                        for kt in range(NT):
                            c = dpool.tile([P, TQ], f32, tag="dbgc3", name=f"ca{kt}", bufs=1)
                            nc.vector.tensor_copy(c, attnT[:, kt, :])
                            nc.sync.dma_start(out=dT[:, kt, :], in_=c)

                # ---------------- phase 4: O-projection + residual -----------
                with ExitStack() as p4:
                    wpool = p4.enter_context(tc.tile_pool(name="wo_pool", bufs=3))
                    ps_o = p4.enter_context(tc.tile_pool(name="ps_o", bufs=2, space="PSUM"))
                    for o in range(NT):
                        wt = wpool.tile([P, NT, P], bf, tag="wo_t")
                        nc.sync.dma_start(
                            out=wt,
                            in_=wo[:, o * P:(o + 1) * P].rearrange("(n p) m -> p n m", p=P))
                        ps = ps_o.tile([P, TQ], f32, tag="o_ps")
                        for kf in range(NT):
                            nc.tensor.matmul(ps, wt[:, kf, :], attnT[:, kf, :],
                                             start=(kf == 0), stop=(kf == NT - 1))
                        nc.vector.scalar_tensor_tensor(
                            x2T[:, o, :], ps, bo_sb[:, o:o + 1], x_hi[:, o, :],
                            ALU.add, ALU.add)
                    if debug_outputs:
                        nc.sync.dma_start(out=dbg["d_x2T"].rearrange("(n p) t -> p n t", p=P), in_=x2T)

            # ---------------- phase 5: LN2 + MLP ----------------------------
            with ExitStack() as p5:
                stat = p5.enter_context(tc.tile_pool(name="stat2", bufs=1))
                tmpp = p5.enter_context(tc.tile_pool(name="ln2_tmp", bufs=3))
                h2p = p5.enter_context(tc.tile_pool(name="h2p", bufs=1))
                mp = p5.enter_context(tc.tile_pool(name="m_pool", bufs=1))
                h2T = h2p.tile([P, NT, TQ], bf, tag="h2T")
                mT = mp.tile([P, NFF, TQ], bf, tag="mT")

                with ExitStack() as p5a:
                    psln2 = p5a.enter_context(tc.tile_pool(name="psln2", bufs=1, space="PSUM"))
                    mu_ps = psln2.tile([P, TQ], f32, tag="ln2_mu")
                    sq_ps = psln2.tile([P, TQ], f32, tag="ln2_sq")
                    for kt in range(NT):
                        xbf = tmpp.tile([P, TQ], bf, tag="x2bf")
                        nc.scalar.activation(xbf, x2T[:, kt, :], AF.Copy)
                        sqbf = tmpp.tile([P, TQ], bf, tag="sq2bf")
                        nc.vector.tensor_mul(sqbf, xbf, xbf)
                        nc.tensor.matmul(mu_ps, ones_bf, xbf,
                                         start=(kt == 0), stop=(kt == NT - 1))
                        nc.tensor.matmul(sq_ps, ones_bf, sqbf,
                                         start=(kt == 0), stop=(kt == NT - 1))
                    mu_sb = stat.tile([P, TQ], f32, tag="mu2_sb")
                    rstd = stat.tile([P, TQ], f32, tag="rstd2")
                    tmp1 = stat.tile([P, TQ], f32, tag="ln2tmp1")
                    nc.scalar.activation(mu_sb, mu_ps, AF.Copy, scale=1.0 / H)
                    nc.vector.tensor_mul(tmp1, mu_sb, mu_sb)
                    nc.vector.scalar_tensor_tensor(tmp1, sq_ps, 1.0 / H, tmp1,
                                                   ALU.mult, ALU.subtract)
                    nc.scalar.activation(tmp1, tmp1, AF.Sqrt, bias=eps_sb)
                    nc.vector.reciprocal(rstd, tmp1)
                    for kt in range(NT):
                        t = tmpp.tile([P, TQ], f32, tag="h2tmp")
                        nc.vector.tensor_sub(t, x2T[:, kt, :], mu_sb)
                        nc.vector.tensor_mul(t, t, rstd)
                        nc.scalar.activation(h2T[:, kt, :], t, AF.Identity,
                                             bias=ln_sb[:, 3 * NT + kt:3 * NT + kt + 1],
                                             scale=ln_sb[:, 2 * NT + kt:2 * NT + kt + 1])

                # ------------- phase 6: MLP fc + gelu ------------------------
                with ExitStack() as p6:
                    wpool = p6.enter_context(tc.tile_pool(name="wfc_pool", bufs=2))
                    psfc = p6.enter_context(tc.tile_pool(name="psfc", bufs=4, space="PSUM"))
                    for ffg in range(8):       # groups of 4 ff-tiles
                        wt = wpool.tile([P, NT, 4 * P], bf, tag="wfc_t")
                        nc.sync.dma_start(
                            out=wt,
                            in_=wfc[:, ffg * 4 * P:(ffg + 1) * 4 * P]
                            .rearrange("(n p) m -> p n m", p=P))
                        for fl in range(4):
                            ff = ffg * 4 + fl
                            ps = psfc.tile([P, TQ], f32, tag="fc_ps")
                            for kt in range(NT):
                                nc.tensor.matmul(
                                    ps, wt[:, kt, fl * P:(fl + 1) * P], h2T[:, kt, :],
                                    start=(kt == 0), stop=(kt == NT - 1))
                            nc.scalar.activation(mT[:, ff, :], ps, AF.Gelu,
                                                 bias=bfc_sb[:, ff:ff + 1])
                    if debug_outputs:
                        dT = dbg["d_mT"].rearrange("(n p) t -> p n t", p=P)
                        for ff in range(NFF):
                            c = wpool.tile([P, TQ], f32, tag="dbgc6")
                            nc.vector.tensor_copy(c, mT[:, ff, :])
                            nc.sync.dma_start(out=dT[:, ff, :], in_=c)

                # --------- phase 7: MLP proj + residual + out ----------------
                with ExitStack() as p7:
                    wpool2 = p7.enter_context(tc.tile_pool(name="wpr_pool", bufs=3))
                    op = p7.enter_context(tc.tile_pool(name="out_pool", bufs=3))
                    pspr = p7.enter_context(tc.tile_pool(name="pspr", bufs=1, space="PSUM"))
                    pr_ps = [pspr.tile([P, TQ], f32, tag=f"pr_ps{o}", name=f"pr_ps{o}")
                             for o in range(NT)]
                    for fk in range(NFF):
                        wt = wpool2.tile([P, H], bf, tag="wpr_t")
                        nc.sync.dma_start(out=wt, in_=wpr[fk * P:(fk + 1) * P, :])
                        for o in range(NT):
                            nc.tensor.matmul(
                                pr_ps[o], wt[:, o * P:(o + 1) * P], mT[:, fk, :],
                                start=(fk == 0), stop=(fk == NFF - 1))
                    for o in range(NT):
                        ot = op.tile([P, TQ], f32, tag="ot")
                        nc.vector.scalar_tensor_tensor(
                            ot, pr_ps[o], bpr_sb[:, o:o + 1], x2T[:, o, :],
                            ALU.add, ALU.add)
                        nc.sync.dma_start(
                            out=outT[o * P:(o + 1) * P, :], in_=ot)

    with tile.TileContext(nc) as tc, ExitStack() as top:
        const1 = top.enter_context(tc.tile_pool(name="const1", bufs=1))
        ones_bf = const1.tile([P, P], bf)
        nc.vector.memset(ones_bf, 1.0)
        if repeat == 1:
            body(tc, const1, ones_bf)
        else:
            engs = (mybir.EngineType.PE, mybir.EngineType.DVE,
                    mybir.EngineType.Activation, mybir.EngineType.SP,
                    mybir.EngineType.Pool)
            with tc.For_i(0, repeat, 1, hint_engines=engs):
                body(tc, const1, ones_bf)

    nc.compile()
    return nc


# ----------------------------------------------------------------------------
# host-side input preparation
# ----------------------------------------------------------------------------

def _rope_tables():
    half = HS // 2
    inv_freq = 1.0 / (10000.0 ** (np.arange(half, dtype=np.float32) / half))
    t = np.arange(T, dtype=np.float32)
    ang = t[None, :] * inv_freq[(np.arange(P) % half)][:, None]   # [128, T]
    cos = np.cos(ang).astype(np.float32)
    sin = np.sin(ang).astype(np.float32)
    # ssgn rows: +sin for j=0 rows (p%64<32), -sin for j=1 rows
    sgn = np.where((np.arange(P) % HS) < half, 1.0, -1.0).astype(np.float32)
    ssgn = sin * sgn[:, None]
    return cos, ssgn


def _perm():
    # new pos (hd, j, i) <- old feature hd*64 + 2i + j
    idx = np.arange(H).reshape(NH, HS // 2, 2)
    return idx.transpose(0, 2, 1).reshape(H)


def _col_tiles(v):
    # [N] -> [128, N//128] with column j = v[j*128:(j+1)*128]
    return np.ascontiguousarray(v.reshape(-1, P).T).astype(np.float32)


def prepare_in_maps(inputs):
    x = np.asarray(inputs["x"], np.float32)
    deint = _perm()
    wq_ = np.asarray(inputs["Wq"], np.float32)[:, deint].astype(_bf16)
    wk_ = np.asarray(inputs["Wk"], np.float32)[:, deint].astype(_bf16)
    wv_ = np.asarray(inputs["Wv"], np.float32).astype(_bf16)
    wo_ = np.asarray(inputs["Wo"], np.float32).astype(_bf16)
    wfc_ = np.asarray(inputs["Wfc"], np.float32).astype(_bf16)
    wpr_ = np.asarray(inputs["Wpr"], np.float32).astype(_bf16)
    cos, ssgn = _rope_tables()

    ql = np.arange(TQ)
    mask_hi = np.zeros((P, 4, TQ), np.float32)
    for j in range(4):
        mask_hi[:, j, :] = (j * P + np.arange(P)[:, None]) <= ql[None, :]
    mask_hi = mask_hi.astype(_bf16)

    shared = dict(
        wq=wq_, wk=wk_, wv=wv_, wo=wo_, wfc=wfc_, wpr=wpr_,
        bq=_col_tiles(np.asarray(inputs["bq"], np.float32)[deint]),
        bk=_col_tiles(np.asarray(inputs["bk"], np.float32)[deint]),
        bo=_col_tiles(np.asarray(inputs["bo"], np.float32)),
        bpr=_col_tiles(np.asarray(inputs["bpr"], np.float32)),
        bfc=_col_tiles(np.asarray(inputs["bfc"], np.float32)),
        ln1w=_col_tiles(np.asarray(inputs["ln1_w"], np.float32)),
        ln1b=_col_tiles(np.asarray(inputs["ln1_b"], np.float32)),
        ln2w=_col_tiles(np.asarray(inputs["ln2_w"], np.float32)),
        ln2b=_col_tiles(np.asarray(inputs["ln2_b"], np.float32)),
        bvb=np.broadcast_to(np.asarray(inputs["bv"], np.float32)[None, :], (P, H)).copy(),
        mask_hi=mask_hi,
    )

    in_maps = []
    for c in range(NCORES):
        b, h = c // 2, c % 2
        if h == 0:
            colperm = np.concatenate([np.arange(TQ, T), np.arange(0, TQ)])
        else:
            colperm = np.arange(T)
        xTb = np.ascontiguousarray(x[b].T[:, colperm])       # [H, T] rotated
        m = dict(shared)
        m["xT_lo"] = np.ascontiguousarray(xTb[:, 0:TQ]).astype(_bf16)
        m["xT_hi"] = np.ascontiguousarray(xTb[:, TQ:T])
        m["cosK"] = np.ascontiguousarray(cos[:, colperm]).astype(_bf16)
        m["ssgnK"] = np.ascontiguousarray(ssgn[:, colperm]).astype(_bf16)
        m["mask_lo"] = np.full((P, TQ), 0.0 if h == 0 else 1.0, _bf16)
        m["mscal"] = np.full((P, 1), 0.0 if h == 0 else 1.0, np.float32)
        in_maps.append(m)
    return in_maps


def gather(results):
    out = np.empty((B, T, H), np.float32)
    for c in range(NCORES):
        b, h = c // 2, c % 2
        out[b, h * TQ:(h + 1) * TQ, :] = results[c]["outT"].T
    return out


# ----------------------------------------------------------------------------
# public entry point
# ----------------------------------------------------------------------------

_NC = None


def kernel(**inputs):
    global _NC
    from concourse.bass_utils import run_bass_kernel_spmd
    if _NC is None:
        _NC = build(repeat=1)
    in_maps = prepare_in_maps(inputs)
    res = run_bass_kernel_spmd(_NC, in_maps, list(range(NCORES)))
    return gather(res.results)


# revision 17
# speedup vs baseline: 1.0514x; 1.0514x over previous
"""Trainium2 Bass kernel for a dense transformer block (B=4,T=1024,H=1024,NH=16,FF=4096).

Sharding: 8 cores = (batch b, token-half h). Each core computes the full block
for its 512 query tokens; K/V projections are computed over all 1024 tokens of
the batch on each core (no cross-core collectives).

Device layout is fully "transposed": activations live as [feature->partitions,
token->free] SBUF tiles. LayerNorm/softmax reductions over features/keys become
TensorE ones-matmuls (fused reduce+broadcast). Token-half cores are made
SPMD-uniform by rotating core h=0's xT columns by 512 so query tokens are
always xT columns 512:1024; all per-core differences (masks, RoPE tables) are
inputs. The reference's softmax-then-multiplicative-mask semantics are kept:
exp over all keys feeds the denominator, masked exp feeds the AV matmul.
"""
import sys
sys.path.insert(0, "/opt/trn_rl_repo")
import numpy as np
import ml_dtypes

B, T, H, NH = 4, 1024, 1024, 16
HS = H // NH          # 64
FF = 4 * H            # 4096
EPS = 1e-5
P = 128
TQ = T // 2           # 512 query tokens per core
NT = T // P           # 8 feature/token tiles
NFF = FF // P         # 32
NCORES = 8

_bf16 = ml_dtypes.bfloat16


# ----------------------------------------------------------------------------
# device program
# ----------------------------------------------------------------------------

def build(repeat=1, debug_outputs=False):
    import concourse.bass as bass
    import concourse.mybir as mybir
    import concourse.tile as tile
    from concourse import bacc
    from contextlib import ExitStack

    f32 = mybir.dt.float32
    bf = mybir.dt.bfloat16
    AF = mybir.ActivationFunctionType
    ALU = mybir.AluOpType

    nc = bacc.Bacc("TRN2", target_bir_lowering=False, debug=False,
                   num_devices=NCORES)

    def din(name, shape, dt=f32):
        return nc.dram_tensor(name, shape, dt, kind="ExternalInput").ap()

    # per-core inputs
    xT_lo = din("xT_lo", [H, TQ], bf)        # x^T columns 0:512 (bf16)
    xT_hi = din("xT_hi", [H, TQ])            # x^T columns 512:1024 (= query tokens)
    wq = din("wq", [H, H], bf)               # [h_in, f_out], rope-permuted cols
    wk = din("wk", [H, H], bf)
    wv = din("wv", [H, H], bf)
    wo = din("wo", [H, H], bf)
    wfc = din("wfc", [H, FF], bf)
    wpr = din("wpr", [FF, H], bf)
    bq = din("bq", [P, NT])                  # permuted, [partition, tile]
    bk = din("bk", [P, NT])
    bo = din("bo", [P, NT])
    bpr = din("bpr", [P, NT])
    bfc = din("bfc", [P, NFF])
    ln1w = din("ln1w", [P, NT])
    ln1b = din("ln1b", [P, NT])
    ln2w = din("ln2w", [P, NT])
    ln2b = din("ln2b", [P, NT])
    bvb = din("bvb", [P, H])                 # bv broadcast across partitions
    cosK = din("cosK", [P, T], bf)           # rope tables, xT column order
    ssgnK = din("ssgnK", [P, T], bf)         # +sin rows j=0, -sin rows j=1
    mask_lo = din("mask_lo", [P, TQ], bf)    # kt 0..3 mask (all-0 or all-1)
    mscal = din("mscal", [P, 1])             # kt 0..3 mask as scalar (0.0 or 1.0)
    mask_hi = din("mask_hi", [P, 4, TQ], bf) # kt 4..7 triangular masks

    outT = nc.dram_tensor("outT", [H, TQ], f32, kind="ExternalOutput").ap()
    dbg = {}
    if debug_outputs:
        for name, shape in [("d_h1T", [H, T]), ("d_kT", [H, T]), ("d_qT", [H, TQ]),
                            ("d_v", [T, H]), ("d_attnT", [H, TQ]),
                            ("d_x2T", [H, TQ]), ("d_mT", [FF, TQ])]:
            dbg[name] = nc.dram_tensor(name, shape, f32, kind="ExternalOutput").ap()

    def body(tc, const1, ones_bf):
        with ExitStack() as ctx:
            # ------------ long-lived pools for this block iteration ----------
            persist = ctx.enter_context(tc.tile_pool(name="persist", bufs=1))
            x_hi = persist.tile([P, NT, TQ], f32, tag="x_hi")
            x2T = persist.tile([P, NT, TQ], f32, tag="x2T")
            biases = persist.tile([P, 2 * NT], f32, tag="biases")  # bq|bk
            bo_sb = persist.tile([P, NT], f32, tag="bo_sb")
            bpr_sb = persist.tile([P, NT], f32, tag="bpr_sb")
            bfc_sb = persist.tile([P, NFF], f32, tag="bfc_sb")
            ln_sb = persist.tile([P, 4 * NT], f32, tag="ln_sb")  # ln1w|ln1b|ln2w|ln2b
            eps_sb = persist.tile([P, 1], f32, tag="eps_sb")
            nc.vector.memset(eps_sb, EPS)
            msc_sb = persist.tile([P, 1], f32, tag="msc_sb")
            nc.sync.dma_start(out=msc_sb, in_=mscal)

            nc.sync.dma_start(out=biases[:, 0:NT], in_=bq)
            nc.sync.dma_start(out=biases[:, NT:2 * NT], in_=bk)
            nc.sync.dma_start(out=bo_sb, in_=bo)
            nc.sync.dma_start(out=bpr_sb, in_=bpr)
            nc.sync.dma_start(out=bfc_sb, in_=bfc)
            nc.sync.dma_start(out=ln_sb[:, 0:NT], in_=ln1w)
            nc.sync.dma_start(out=ln_sb[:, NT:2 * NT], in_=ln1b)
            nc.sync.dma_start(out=ln_sb[:, 2 * NT:3 * NT], in_=ln2w)
            nc.sync.dma_start(out=ln_sb[:, 3 * NT:4 * NT], in_=ln2b)
            nc.sync.dma_start(out=x_hi, in_=xT_hi.rearrange("(n p) t -> p n t", p=P))

            with ExitStack() as attn_scope:
                aacts = attn_scope.enter_context(tc.tile_pool(name="aacts", bufs=1))
                h1T = aacts.tile([P, NT, T], bf, tag="h1T")
                kT = aacts.tile([P, NT, T], bf, tag="kT")
                qT = aacts.tile([P, NT, TQ], bf, tag="qT")
                vsb = aacts.tile([P, NT, H], bf, tag="vsb")     # [tok_p, kt, feat]
                attnT = aacts.tile([P, NT, TQ], bf, tag="attnT")
                # [V'|ones] fused stationary operand for kt 0..3:
                # layout [p, kt, head, {V',ones}, hs]
                vaug = aacts.tile([P, 4, NH, 2, HS], bf, tag="vaug")
                bvb_sb = aacts.tile([P, H], f32, tag="bvb_sb")
                cos_sb = aacts.tile([P, T], bf, tag="cos_sb")
                ssgn_sb = aacts.tile([P, T], bf, tag="ssgn_sb")
                mhi_sb = aacts.tile([P, 4, TQ], bf, tag="mhi_sb")

                nc.sync.dma_start(out=bvb_sb, in_=bvb)
                nc.sync.dma_start(out=cos_sb, in_=cosK)
                nc.sync.dma_start(out=ssgn_sb, in_=ssgnK)
                nc.sync.dma_start(out=mhi_sb, in_=mask_hi)

                # ---------------- phase 1: LN1 (transposed layernorm) --------
                with ExitStack() as p1:
                    x_lo_p = p1.enter_context(tc.tile_pool(name="x_lo", bufs=1))
                    stat = p1.enter_context(tc.tile_pool(name="stat", bufs=1))
                    tmpp = p1.enter_context(tc.tile_pool(name="ln_tmp", bufs=3))
                    psln = p1.enter_context(tc.tile_pool(name="psln", bufs=1, space="PSUM"))

                    x_lo = x_lo_p.tile([P, NT, TQ], bf, tag="x_lo")
                    nc.sync.dma_start(out=x_lo, in_=xT_lo.rearrange("(n p) t -> p n t", p=P))

                    mu_ps = psln.tile([P, T], f32, tag="ln_mu")
                    sq_ps = psln.tile([P, T], f32, tag="ln_sq")
                    for kt in range(NT):
                        xbf_hi = tmpp.tile([P, TQ], bf, tag="xbf_hi")
                        nc.scalar.activation(xbf_hi, x_hi[:, kt, :], AF.Copy)
                        sq_lo = tmpp.tile([P, TQ], bf, tag="sq_lo")
                        nc.vector.tensor_mul(sq_lo, x_lo[:, kt, :], x_lo[:, kt, :])
                        sq_hi = tmpp.tile([P, TQ], bf, tag="sq_hi")
                        nc.vector.tensor_mul(sq_hi, xbf_hi, xbf_hi)
                        first, last = (kt == 0), (kt == NT - 1)
                        nc.tensor.matmul(mu_ps[:, 0:TQ], ones_bf, x_lo[:, kt, :],
                                         start=first, stop=last)
                        nc.tensor.matmul(mu_ps[:, TQ:T], ones_bf, xbf_hi,
                                         start=first, stop=last)
                        nc.tensor.matmul(sq_ps[:, 0:TQ], ones_bf, sq_lo,
                                         start=first, stop=last)
                        nc.tensor.matmul(sq_ps[:, TQ:T], ones_bf, sq_hi,
                                         start=first, stop=last)

                    mu_sb = stat.tile([P, T], f32, tag="mu_sb")
                    rstd = stat.tile([P, T], f32, tag="rstd")
                    tmp1 = stat.tile([P, T], f32, tag="lntmp1")
                    nc.scalar.activation(mu_sb, mu_ps, AF.Copy, scale=1.0 / H)
                    nc.vector.tensor_mul(tmp1, mu_sb, mu_sb)
                    nc.vector.scalar_tensor_tensor(tmp1, sq_ps, 1.0 / H, tmp1,
                                                   ALU.mult, ALU.subtract)
                    nc.scalar.activation(tmp1, tmp1, AF.Sqrt, bias=eps_sb)
                    nc.vector.reciprocal(rstd, tmp1)

                    for kt in range(NT):
                        for blk, xpart in ((0, x_lo[:, kt, :]), (1, x_hi[:, kt, :])):
                            sl = slice(blk * TQ, (blk + 1) * TQ)
                            t = tmpp.tile([P, TQ], f32, tag="h1tmp")
                            nc.vector.tensor_sub(t, xpart, mu_sb[:, sl])
                            nc.vector.tensor_mul(t, t, rstd[:, sl])
                            nc.scalar.activation(h1T[:, kt, sl], t, AF.Identity,
                                                 bias=ln_sb[:, NT + kt:NT + kt + 1],
                                                 scale=ln_sb[:, kt:kt + 1])
                    if debug_outputs:
                        dT = dbg["d_h1T"].rearrange("(n p) t -> p n t", p=P)
                        for kt in range(NT):
                            c = tmpp.tile([P, T], f32, tag="dbgc")
                            nc.vector.tensor_copy(c, h1T[:, kt, :])
                            nc.sync.dma_start(out=dT[:, kt, :], in_=c)

                # ---------------- phase 2: QKV projections + RoPE ------------
                with ExitStack() as p2:
                    wpool = p2.enter_context(tc.tile_pool(name="wqkv", bufs=3))
                    rtmp = p2.enter_context(tc.tile_pool(name="rtmp", bufs=3))
                    psqkv = p2.enter_context(tc.tile_pool(name="psqkv", bufs=3, space="PSUM"))

                    # V in token-partition layout: [tok, feat]
                    wv_sb = wpool.tile([P, NT, H], bf, tag="wv_all", bufs=1)
                    nc.sync.dma_start(out=wv_sb, in_=wv.rearrange("(n p) m -> p n m", p=P))
                    for tt in range(NT):
                        for fb in range(2):
                            fsl = slice(fb * TQ, (fb + 1) * TQ)
                            ps = psqkv.tile([P, TQ], f32, tag="qkv_ps", name=f"vps{tt}_{fb}")
                            for kt in range(NT):
                                nc.tensor.matmul(
                                    ps, h1T[:, kt, tt * P:(tt + 1) * P], wv_sb[:, kt, fsl],
                                    start=(kt == 0), stop=(kt == NT - 1))
                            nc.vector.tensor_add(vsb[:, tt, fsl], ps, bvb_sb[:, fsl])

                    nc.gpsimd.memset(vaug[:, :, :, 1, :], 1.0)
                    for kt in range(4):
                        nc.vector.tensor_scalar_mul(
                            vaug[:, kt, :, 0, :],
                            vsb[:, kt, :].rearrange("p (h d) -> p h d", h=NH),
                            msc_sb)

                    for which in (0, 1):
                        wdram = wk if which == 0 else wq
                        bias_off = NT if which == 0 else 0
                        cols = slice(0, T) if which == 0 else slice(TQ, T)
                        nblk = (cols.stop - cols.start) // TQ
                        for fo in range(NT):
                            wt = wpool.tile([P, NT, P], bf, tag="wqkv", name=f"w{which}_{fo}")
                            nc.sync.dma_start(
                                out=wt,
                                in_=wdram[:, fo * P:(fo + 1) * P]
                                .rearrange("(n p) m -> p n m", p=P))
                            for blk in range(nblk):
                                sl = slice(cols.start + blk * TQ, cols.start + (blk + 1) * TQ)
                                osl = slice(blk * TQ, (blk + 1) * TQ)
                                ps = psqkv.tile([P, TQ], f32, tag="qkv_ps", name=f"qkp{which}_{fo}_{blk}")
                                for kt in range(NT):
                                    nc.tensor.matmul(ps, wt[:, kt, :], h1T[:, kt, sl],
                                                     start=(kt == 0), stop=(kt == NT - 1))
                                # rope: r = (ps+b)*cos + swap32((ps+b)*ssgn)
                                braw = rtmp.tile([P, TQ], bf, tag="braw", name=f"braw{which}_{fo}_{blk}")
                                bcol = biases[:, bias_off + fo:bias_off + fo + 1]
                                nc.scalar.activation(braw, ps, AF.Identity, bias=bcol)
                                t1 = rtmp.tile([P, TQ], bf, tag="ropet1", name=f"t1_{which}_{fo}_{blk}")
                                t2 = rtmp.tile([P, TQ], bf, tag="ropet2", name=f"t2_{which}_{fo}_{blk}")
                                nc.vector.tensor_mul(t1, braw, ssgn_sb[:, sl])
                                nc.vector.tensor_mul(t2, braw, cos_sb[:, sl])
                                t1s = rtmp.tile([P, TQ], bf, tag="ropet1s", name=f"t1s_{which}_{fo}_{blk}")
                                for q in range(4):
                                    src = q * 32 + (32 if q % 2 == 0 else -32)
                                    nc.sync.dma_start(
                                        out=t1s[q * 32:(q + 1) * 32, :],
                                        in_=t1[src:src + 32, :])
                                dst = kT[:, fo, osl] if which == 0 else qT[:, fo, osl]
                                nc.vector.tensor_add(dst, t2, t1s)

                    if debug_outputs:
                        for nm, srcT, width in (("d_kT", kT, T), ("d_qT", qT, TQ),
                                                ("d_v", vsb, H)):
                            dT = dbg[nm].rearrange("(n p) t -> p n t", p=P)
                            for kt in range(NT):
                                c = rtmp.tile([P, T], f32, tag="dbgc2", name=f"c{nm}_{kt}", bufs=1)
                                nc.vector.tensor_copy(c[:, 0:width], srcT[:, kt, :])
                                nc.sync.dma_start(out=dT[:, kt, :], in_=c[:, 0:width])

                # ---------------- phase 3: attention per head-pair -----------
                with ExitStack() as p3:
                    epool = p3.enter_context(tc.tile_pool(name="epool", bufs=2))
                    dpool = p3.enter_context(tc.tile_pool(name="dpool", bufs=2))
                    ps_s = p3.enter_context(tc.tile_pool(name="ps_s", bufs=3, space="PSUM"))
                    ps_av = p3.enter_context(tc.tile_pool(name="ps_av", bufs=2, space="PSUM"))
                    for hp in range(NH // 2):
                        hd0, hd1 = 2 * hp, 2 * hp + 1
                        ft = hp
                        e_pair = epool.tile([P, NT, 2, TQ], bf, tag="e_pair", name=f"ep{hp}")
                        # row-packed score matmuls for the head pair (K=64 each)
                        for kt in range(NT):
                            ks = slice(kt * P, (kt + 1) * P)
                            sp = ps_s.tile([P, 2, TQ], f32, tag="sp", name=f"sp{hp}_{kt}")
                            nc.tensor.matmul(sp[:, 0, :], kT[0:HS, ft, ks],
                                             qT[0:HS, ft, :],
                                             start=True, stop=True,
                                             tile_position=(0, 0))
                            nc.tensor.matmul(sp[:, 1, :], kT[HS:P, ft, ks],
                                             qT[HS:P, ft, :],
                                             start=True, stop=True,
                                             tile_position=(64, 0))
                            nc.scalar.activation(e_pair[:, kt, :, :], sp, AF.Exp,
                                                 scale=1.0 / np.sqrt(HS))
                        for hd, j in ((hd0, 0), (hd1, 1)):
                            fp = (hd % 2) * HS
                            av2 = ps_av.tile([P, TQ], f32, tag="av2", name=f"av2_{hd}")
                            # kt 0..3: fused [V'|ones] -> rows 0:64 AV, 64:128 D
                            for kt in range(4):
                                nc.tensor.matmul(av2, vaug[:, kt, hd, :, :],
                                                 e_pair[:, kt, j, :],
                                                 start=(kt == 0), stop=False,
                                                 skip_group_check=True)
                            # kt 4..7: col-packed D (unmasked) + AV (masked)
                            em = epool.tile([P, 4, TQ], bf, tag="em", name=f"em{hd}")
                            for i in range(4):
                                nc.vector.tensor_mul(em[:, i, :],
                                                     e_pair[:, 4 + i, j, :],
                                                     mhi_sb[:, i, :])
                            for i in range(4):
                                kt = 4 + i
                                nc.tensor.matmul(av2[HS:P, :], ones_bf[:, 0:HS],
                                                 e_pair[:, kt, j, :],
                                                 start=False, stop=False,
                                                 tile_position=(0, 64),
                                                 skip_group_check=True)
                                nc.tensor.matmul(av2[0:HS, :],
                                                 vsb[:, kt, hd * HS:(hd + 1) * HS],
                                                 em[:, i, :],
                                                 start=False, stop=(kt == NT - 1),
                                                 tile_position=(0, 0),
                                                 skip_group_check=True)
                            rec = dpool.tile([P, TQ], f32, tag="rec", name=f"rec{hd}")
                            nc.vector.reciprocal(rec[0:HS, :], av2[HS:P, :])
                            nc.vector.tensor_mul(attnT[fp:fp + HS, ft, :],
                                                 av2[0:HS, :], rec[0:HS, :])
                    if debug_outputs:
                        dT = dbg["d_attnT"].rearrange("(n p) t -> p n t", p=P)
                        for kt in range(NT):
                            c = dpool.tile([P, TQ], f32, tag="dbgc3", name=f"ca{kt}")
                            nc.vector.tensor_copy(c, attnT[:, kt, :])
                            nc.sync.dma_start(out=dT[:, kt, :], in_=c)

                # ---------------- phase 4: O-projection + residual -----------
                with ExitStack() as p4:
                    wpool = p4.enter_context(tc.tile_pool(name="wo_pool", bufs=3))
                    ps_o = p4.enter_context(tc.tile_pool(name="ps_o", bufs=2, space="PSUM"))
                    for o in range(NT):
                        wt = wpool.tile([P, NT, P], bf, tag="wo_t")
                        nc.sync.dma_start(
                            out=wt,
                            in_=wo[:, o * P:(o + 1) * P].rearrange("(n p) m -> p n m", p=P))
                        ps = ps_o.tile([P, TQ], f32, tag="o_ps")
                        for kf in range(NT):
                            nc.tensor.matmul(ps, wt[:, kf, :], attnT[:, kf, :],
                                             start=(kf == 0), stop=(kf == NT - 1))
                        nc.vector.scalar_tensor_tensor(
                            x2T[:, o, :], ps, bo_sb[:, o:o + 1], x_hi[:, o, :],
                            ALU.add, ALU.add)
                    if debug_outputs:
                        nc.sync.dma_start(out=dbg["d_x2T"].rearrange("(n p) t -> p n t", p=P), in_=x2T)

            # ---------------- phase 5: LN2 + MLP ----------------------------
            with ExitStack() as p5:
                stat = p5.enter_context(tc.tile_pool(name="stat2", bufs=1))
                tmpp = p5.enter_context(tc.tile_pool(name="ln2_tmp", bufs=3))
                h2p = p5.enter_context(tc.tile_pool(name="h2p", bufs=1))
                mp = p5.enter_context(tc.tile_pool(name="m_pool", bufs=1))
                h2T = h2p.tile([P, NT, TQ], bf, tag="h2T")
                mT = mp.tile([P, NFF, TQ], bf, tag="mT")

                with ExitStack() as p5a:
                    psln2 = p5a.enter_context(tc.tile_pool(name="psln2", bufs=1, space="PSUM"))
                    mu_ps = psln2.tile([P, TQ], f32, tag="ln2_mu")
                    sq_ps = psln2.tile([P, TQ], f32, tag="ln2_sq")
                    for kt in range(NT):
                        xbf = tmpp.tile([P, TQ], bf, tag="x2bf")
                        nc.scalar.activation(xbf, x2T[:, kt, :], AF.Copy)
                        sqbf = tmpp.tile([P, TQ], bf, tag="sq2bf")
                        nc.vector.tensor_mul(sqbf, xbf, xbf)
                        nc.tensor.matmul(mu_ps, ones_bf, xbf,
                                         start=(kt == 0), stop=(kt == NT - 1))
                        nc.tensor.matmul(sq_ps, ones_bf, sqbf,
                                         start=(kt == 0), stop=(kt == NT - 1))
                    mu_sb = stat.tile([P, TQ], f32, tag="mu2_sb")
                    rstd = stat.tile([P, TQ], f32, tag="rstd2")
                    tmp1 = stat.tile([P, TQ], f32, tag="ln2tmp1")
                    nc.scalar.activation(mu_sb, mu_ps, AF.Copy, scale=1.0 / H)
                    nc.vector.tensor_mul(tmp1, mu_sb, mu_sb)
                    nc.vector.scalar_tensor_tensor(tmp1, sq_ps, 1.0 / H, tmp1,
                                                   ALU.mult, ALU.subtract)
                    nc.scalar.activation(tmp1, tmp1, AF.Sqrt, bias=eps_sb)
                    nc.vector.reciprocal(rstd, tmp1)
                    for kt in range(NT):
                        t = tmpp.tile([P, TQ], f32, tag="h2tmp")
                        nc.vector.tensor_sub(t, x2T[:, kt, :], mu_sb)
                        nc.vector.tensor_mul(t, t, rstd)
                        nc.scalar.activation(h2T[:, kt, :], t, AF.Identity,
                                             bias=ln_sb[:, 3 * NT + kt:3 * NT + kt + 1],
                                             scale=ln_sb[:, 2 * NT + kt:2 * NT + kt + 1])

                # ------------- phase 6: MLP fc + gelu ------------------------
                with ExitStack() as p6:
                    wpool = p6.enter_context(tc.tile_pool(name="wfc_pool", bufs=2))
                    psfc = p6.enter_context(tc.tile_pool(name="psfc", bufs=4, space="PSUM"))
                    for ffg in range(8):       # groups of 4 ff-tiles
                        wt = wpool.tile([P, NT, 4 * P], bf, tag="wfc_t")
                        nc.sync.dma_start(
                            out=wt,
                            in_=wfc[:, ffg * 4 * P:(ffg + 1) * 4 * P]
                            .rearrange("(n p) m -> p n m", p=P))
                        for fl in range(4):
                            ff = ffg * 4 + fl
                            ps = psfc.tile([P, TQ], f32, tag="fc_ps")
                            for kt in range(NT):
                                nc.tensor.matmul(
                                    ps, wt[:, kt, fl * P:(fl + 1) * P], h2T[:, kt, :],
                                    start=(kt == 0), stop=(kt == NT - 1))
                            nc.scalar.activation(mT[:, ff, :], ps, AF.Gelu,
                                                 bias=bfc_sb[:, ff:ff + 1])
                    if debug_outputs:
                        dT = dbg["d_mT"].rearrange("(n p) t -> p n t", p=P)
                        for ff in range(NFF):
                            c = wpool.tile([P, TQ], f32, tag="dbgc6")
                            nc.vector.tensor_copy(c, mT[:, ff, :])
                            nc.sync.dma_start(out=dT[:, ff, :], in_=c)

                # --------- phase 7: MLP proj + residual + out ----------------
                with ExitStack() as p7:
                    wpool2 = p7.enter_context(tc.tile_pool(name="wpr_pool", bufs=3))
                    op = p7.enter_context(tc.tile_pool(name="out_pool", bufs=3))
                    pspr = p7.enter_context(tc.tile_pool(name="pspr", bufs=1, space="PSUM"))
                    pr_ps = [pspr.tile([P, TQ], f32, tag=f"pr_ps{o}", name=f"pr_ps{o}")
                             for o in range(NT)]
                    for fk in range(NFF):
                        wt = wpool2.tile([P, H], bf, tag="wpr_t")
                        nc.sync.dma_start(out=wt, in_=wpr[fk * P:(fk + 1) * P, :])
                        for o in range(NT):
                            nc.tensor.matmul(
                                pr_ps[o], wt[:, o * P:(o + 1) * P], mT[:, fk, :],
                                start=(fk == 0), stop=(fk == NFF - 1))
                    for o in range(NT):
                        ot = op.tile([P, TQ], f32, tag="ot")
                        nc.vector.scalar_tensor_tensor(
                            ot, pr_ps[o], bpr_sb[:, o:o + 1], x2T[:, o, :],
                            ALU.add, ALU.add)
                        nc.sync.dma_start(
                            out=outT[o * P:(o + 1) * P, :], in_=ot)

    with tile.TileContext(nc) as tc, ExitStack() as top:
        const1 = top.enter_context(tc.tile_pool(name="const1", bufs=1))
        ones_bf = const1.tile([P, P], bf)
        nc.vector.memset(ones_bf, 1.0)
        if repeat == 1:
            body(tc, const1, ones_bf)
        else:
            engs = (mybir.EngineType.PE, mybir.EngineType.DVE,
                    mybir.EngineType.Activation, mybir.EngineType.SP,
                    mybir.EngineType.Pool)
            with tc.For_i(0, repeat, 1, hint_engines=engs):
                body(tc, const1, ones_bf)

    nc.compile()
    return nc


# ----------------------------------------------------------------------------
# host-side input preparation
# ----------------------------------------------------------------------------

def _rope_tables():
    half = HS // 2
    inv_freq = 1.0 / (10000.0 ** (np.arange(half, dtype=np.float32) / half))
    t = np.arange(T, dtype=np.float32)
    ang = t[None, :] * inv_freq[(np.arange(P) % half)][:, None]   # [128, T]
    cos = np.cos(ang).astype(np.float32)
    sin = np.sin(ang).astype(np.float32)
    # ssgn rows: +sin for j=0 rows (p%64<32), -sin for j=1 rows
    sgn = np.where((np.arange(P) % HS) < half, 1.0, -1.0).astype(np.float32)
    ssgn = sin * sgn[:, None]
    return cos, ssgn


def _perm():
    # new pos (hd, j, i) <- old feature hd*64 + 2i + j
    idx = np.arange(H).reshape(NH, HS // 2, 2)
    return idx.transpose(0, 2, 1).reshape(H)


def _col_tiles(v):
    # [N] -> [128, N//128] with column j = v[j*128:(j+1)*128]
    return np.ascontiguousarray(v.reshape(-1, P).T).astype(np.float32)


def prepare_in_maps(inputs):
    x = np.asarray(inputs["x"], np.float32)
    deint = _perm()
    wq_ = np.asarray(inputs["Wq"], np.float32)[:, deint].astype(_bf16)
    wk_ = np.asarray(inputs["Wk"], np.float32)[:, deint].astype(_bf16)
    wv_ = np.asarray(inputs["Wv"], np.float32).astype(_bf16)
    wo_ = np.asarray(inputs["Wo"], np.float32).astype(_bf16)
    wfc_ = np.asarray(inputs["Wfc"], np.float32).astype(_bf16)
    wpr_ = np.asarray(inputs["Wpr"], np.float32).astype(_bf16)
    cos, ssgn = _rope_tables()

    ql = np.arange(TQ)
    mask_hi = np.zeros((P, 4, TQ), np.float32)
    for j in range(4):
        mask_hi[:, j, :] = (j * P + np.arange(P)[:, None]) <= ql[None, :]
    mask_hi = mask_hi.astype(_bf16)

    shared = dict(
        wq=wq_, wk=wk_, wv=wv_, wo=wo_, wfc=wfc_, wpr=wpr_,
        bq=_col_tiles(np.asarray(inputs["bq"], np.float32)[deint]),
        bk=_col_tiles(np.asarray(inputs["bk"], np.float32)[deint]),
        bo=_col_tiles(np.asarray(inputs["bo"], np.float32)),
        bpr=_col_tiles(np.asarray(inputs["bpr"], np.float32)),
        bfc=_col_tiles(np.asarray(inputs["bfc"], np.float32)),
        ln1w=_col_tiles(np.asarray(inputs["ln1_w"], np.float32)),
        ln1b=_col_tiles(np.asarray(inputs["ln1_b"], np.float32)),
        ln2w=_col_tiles(np.asarray(inputs["ln2_w"], np.float32)),
        ln2b=_col_tiles(np.asarray(inputs["ln2_b"], np.float32)),
        bvb=np.broadcast_to(np.asarray(inputs["bv"], np.float32)[None, :], (P, H)).copy(),
        mask_hi=mask_hi,
    )

    in_maps = []
    for c in range(NCORES):
        b, h = c // 2, c % 2
        if h == 0:
            colperm = np.concatenate([np.arange(TQ, T), np.arange(0, TQ)])
        else:
            colperm = np.arange(T)
        xTb = np.ascontiguousarray(x[b].T[:, colperm])       # [H, T] rotated
        m = dict(shared)
        m["xT_lo"] = np.ascontiguousarray(xTb[:, 0:TQ]).astype(_bf16)
        m["xT_hi"] = np.ascontiguousarray(xTb[:, TQ:T])
        m["cosK"] = np.ascontiguousarray(cos[:, colperm]).astype(_bf16)
        m["ssgnK"] = np.ascontiguousarray(ssgn[:, colperm]).astype(_bf16)
        m["mask_lo"] = np.full((P, TQ), 0.0 if h == 0 else 1.0, _bf16)
        m["mscal"] = np.full((P, 1), 0.0 if h == 0 else 1.0, np.float32)
        in_maps.append(m)
    return in_maps


def gather(results):
    out = np.empty((B, T, H), np.float32)
    for c in range(NCORES):
        b, h = c // 2, c % 2
        out[b, h * TQ:(h + 1) * TQ, :] = results[c]["outT"].T
    return out


# ----------------------------------------------------------------------------
# public entry point
# ----------------------------------------------------------------------------

_NC = None


def kernel(**inputs):
    global _NC
    from concourse.bass_utils import run_bass_kernel_spmd
    if _NC is None:
        _NC = build(repeat=1)
    in_maps = prepare_in_maps(inputs)
    res = run_bass_kernel_spmd(_NC, in_maps, list(range(NCORES)))
    return gather(res.results)
